# revision 9
# baseline (speedup 1.0000x reference)
"""Self-contained Trainium2 kernel for nn_ConViTCast_20315195310358.

Sharding: 8 cores = 2 batch samples x 4 token-quarters (128 tokens/core).
All compute is token-parallel except block attention, which all-gathers
K^T and V within each 4-core group (2 AllGathers per depth).
Precision: fp32 state; fp32r (TF32-like) matmuls for projections/head;
bf16 matmuls inside the RK4 ODE (errors damped by RES/6 accumulation)
and for the gathered attention K/V/exp tensors.
"""

import contextlib
import sys

import numpy as np

sys.path.insert(0, "/opt/trn_rl_repo")

import ml_dtypes
import concourse.bacc as bacc
import concourse.tile as tile
from concourse import mybir
from concourse import bass_utils

F32 = mybir.dt.float32
F32R = mybir.dt.float32r
BF16 = mybir.dt.bfloat16
AF = mybir.ActivationFunctionType
ALU = mybir.AluOpType
AX = mybir.AxisListType

B, V, H, W, P = 2, 5, 32, 64, 2
D, NH, HD, DEPTH = 1024, 16, 64, 2
L = (H // P) * (W // P)          # 512
RES, NSTEPS, EPS = 0.01, 10, 1e-5
NCORES, GS, TPC = 8, 4, 128
KC = D // 128
SC = HD ** -0.5
NOUT = V * P * P

# offsets in rows_r DRAM tensor [1, 13312] (f32r bias rows)
RR = {"aggkb": 0, "aggvb": 1024, "aggob": 2048, "ob0": 3072, "ob1": 4096,
      "b1b2_0": 5120, "b1b2_1": 7168, "headb": 9216, "hfb": 11264}
RRN = 13312
# offsets in rows_f DRAM tensor [1, 3088] (f32 rows)
FO = {"ltw": 0, "ltb": 1024, "aggqb": 2048, "steps": 3072}
FN = 3088
LN = {"n1_0": 0, "n1b_0": 1, "n2_0": 2, "n2b_0": 3,
      "n1_1": 4, "n1b_1": 5, "n2_1": 6, "n2b_1": 7,
      "og_0": 8, "ob_0": 9, "og_1": 10, "ob_1": 11,
      "normg": 12, "normb": 13}

_CACHE = {}


def build_program():
    nc = bacc.Bacc("TRN2", target_bir_lowering=False, debug=False,
                   num_devices=NCORES)

    def din(name, shape, dt=F32):
        return nc.dram_tensor(name, shape, dt, kind="ExternalInput").ap()

    patches_d = din("patches", [4, V * TPC], F32R)
    lead_d = din("lead", [1, 1])
    pos_d = din("pos", [TPC, D])
    pw_d = din("pw", [4, V * D], F32R)
    aggk_d = din("aggk", [D, D], F32R)
    aggv_d = din("aggv", [D, D], F32R)
    aggq_d = din("aggq", [D, D], F32R)
    aggo_d = din("aggo", [D, D], F32R)
    vq_d = din("vq", [D, 1], F32R)
    qkv_d = [din(f"qkv{i}", [D, 3 * D], F32R) for i in range(DEPTH)]
    ow_d = [din(f"ow{i}", [D, D], F32R) for i in range(DEPTH)]
    w1_d = [din(f"w1_{i}", [D, D], BF16) for i in range(DEPTH)]
    w2_d = [din(f"w2_{i}", [D, D], BF16) for i in range(DEPTH)]
    h1_d = din("h1w", [D, D], F32R)
    h2_d = din("h2w", [D, D], F32R)
    hf_d = din("hfw", [D, NOUT], F32R)
    rows_r_d = din("rows_r", [1, RRN], F32R)
    tokb_d = din("tokb", [1, V * D], F32R)
    qkvb_d = [din(f"qkvb{i}", [1, 3 * D], F32R) for i in range(DEPTH)]
    rows_f_d = din("rows_f", [1, FN])
    ln_d = din("lnrows", [14, D])
    ident_d = din("ident", [128, 128])
    identr_d = din("identr", [128, 128], F32R)
    identb_d = din("identb", [128, 128], BF16)
    ones_d = din("ones_r", [1, 128], F32R)

    y_d = nc.dram_tensor("y", [TPC, NOUT], F32, kind="ExternalOutput").ap()

    rg = [[0, 1, 2, 3], [4, 5, 6, 7]]

    with tile.TileContext(nc) as tc, contextlib.ExitStack() as ctx:
        const = ctx.enter_context(tc.tile_pool(name="const", bufs=1))
        sb2 = ctx.enter_context(tc.tile_pool(name="sb2", bufs=2))
        lnbc = ctx.enter_context(tc.tile_pool(name="lnbc", bufs=4))
        tpool = ctx.enter_context(tc.tile_pool(name="tpool", bufs=2))
        aw = ctx.enter_context(tc.tile_pool(name="aw", bufs=2))
        rowp = ctx.enter_context(tc.tile_pool(name="rowp", bufs=2))
        pt = ctx.enter_context(tc.tile_pool(name="pt", bufs=4, space="PSUM"))
        pm = ctx.enter_context(tc.tile_pool(name="pm", bufs=2, space="PSUM"))
        dram = ctx.enter_context(tc.tile_pool(name="dram", bufs=1,
                                              space="DRAM"))

        ident = const.tile([128, 128], F32)
        nc.sync.dma_start(out=ident[:], in_=ident_d)
        identr = const.tile([128, 128], F32R)
        nc.sync.dma_start(out=identr[:], in_=identr_d)
        identb = const.tile([128, 128], BF16)
        nc.sync.dma_start(out=identb[:], in_=identb_d)
        ones_r = const.tile([1, 128], F32R)
        nc.sync.dma_start(out=ones_r[:], in_=ones_d)
        rows5 = const.tile([1, 5120], F32R)
        nc.sync.dma_start(out=rows5[:], in_=rows_r_d[:, :5120])
        hfw = const.tile([128, KC, NOUT], F32R)
        nc.sync.dma_start(out=hfw[:],
                          in_=hf_d.rearrange("(c p) n -> p c n", p=128))
        mask = const.tile([128, NSTEPS], F32)
        epst = const.tile([128, 1], F32)
        nc.vector.memset(epst[:], EPS)
        xs = const.tile([TPC, D], F32)
        z_t = const.tile([TPC, D], F32)

        def r5(nm, n0, n):
            return rows5[:, RR[nm] + n0:RR[nm] + n0 + n]

        def rload(nm, n, tag):
            t = rowp.tile([1, 2048], F32R, tag="rowp", name=f"row_{tag}")
            nc.sync.dma_start(out=t[:, :n], in_=rows_r_d[:, RR[nm]:RR[nm] + n])
            return t

        def ln_bc(idx, tag):
            t = lnbc.tile([128, D], F32, tag="lnbc", name=f"lnbc_{tag}")
            nc.sync.dma_start(
                out=t[:], in_=ln_d[idx:idx + 1, :].partition_broadcast(128))
            return t

        def load_w(dram_ap, tag, col0, n=512):
            t = aw.tile([128, KC, 512], F32R, tag="aw", name=f"aw_{tag}")
            src = dram_ap.rearrange("(c p) n -> p c n", p=128)
            for c in range(KC):
                nc.sync.dma_start(out=t[:, c, :n], in_=src[:, c, col0:col0 + n])
            return t

        def mm_opt_a(lhs, w_dram, bias_ap, name, extra_row=None):
            """psum [128,1024] = lhs @ W (+bias row +extra row)."""
            pmt = pm.tile([128, 1024], F32, tag="pm", name=name)
            for nb in range(2):
                sl = pmt[:, nb * 512:(nb + 1) * 512]
                wh = load_w(w_dram, f"{name}_{nb}", nb * 512)
                first = True
                if bias_ap is not None:
                    nc.tensor.matmul(sl, ones_r[:],
                                     bias_ap[:, nb * 512:(nb + 1) * 512],
                                     start=True, stop=False)
                    first = False
                if extra_row is not None:
                    nc.tensor.matmul(sl, ones_r[:],
                                     extra_row[:, nb * 512:(nb + 1) * 512],
                                     start=first, stop=False)
                    first = False
                for c in range(KC):
                    nc.tensor.matmul(sl, lhs[:, c, :], wh[:, c, :],
                                     start=(first and c == 0),
                                     stop=(c == KC - 1))
            return pmt

        def transpose_1024(dst, src, idt, tag):
            for half in range(2):
                ps = pt.tile([128, 512], src.dtype, tag="pt",
                             name=f"tp_{tag}{half}")
                for j in range(4):
                    c = half * 4 + j
                    nc.tensor.transpose(ps[:, j * 128:(j + 1) * 128],
                                        src[:, c * 128:(c + 1) * 128], idt)
                nc.vector.tensor_copy(
                    dst[:, half * 4:(half + 1) * 4, :].rearrange(
                        "p c t -> p (c t)"), ps[:])

        def ln_tok(out, z, g_bc, b_bc):
            stats = sb2.tile([128, 2, 6], F32, tag="ln_s", name="ln_s")
            zr = z.rearrange("p (g d) -> p g d", g=2)
            for g in range(2):
                nc.vector.bn_stats(out=stats[:, g, :], in_=zr[:, g, :])
            mv = sb2.tile([128, 2], F32, tag="ln_mv", name="ln_mv")
            nc.vector.bn_aggr(out=mv[:], in_=stats[:])
            std = sb2.tile([128, 1], F32, tag="ln_std", name="ln_std")
            nc.scalar.activation(std[:], mv[:, 1:2], AF.Sqrt, bias=epst[:])
            rstd = sb2.tile([128, 1], F32, tag="ln_rs", name="ln_rs")
            nc.vector.reciprocal(rstd[:], std[:])
            beta = sb2.tile([128, 1], F32, tag="ln_b", name="ln_b")
            nc.vector.tensor_scalar(beta[:], mv[:, 0:1], rstd[:], -1.0,
                                    ALU.mult, ALU.mult)
            nc.vector.tensor_scalar(out, z, rstd[:], beta[:], ALU.mult, ALU.add)
            nc.vector.tensor_mul(out, out, g_bc)
            nc.vector.tensor_add(out, out, b_bc)

        # ================= Phase 1: patch embed + variable aggregation
        with tc.tile_pool(name="aggp", bufs=1) as ag:
            rows_f = ag.tile([1, FN], F32)
            nc.sync.dma_start(out=rows_f[:], in_=rows_f_d)
            lead = ag.tile([1, 1], F32)
            nc.sync.dma_start(out=lead[:], in_=lead_d)
            pat = ag.tile([4, V * TPC], F32R)
            nc.sync.dma_start(out=pat[:], in_=patches_d)
            vq = ag.tile([128, KC, 1], F32R)
            nc.sync.dma_start(out=vq[:],
                              in_=vq_d.rearrange("(c p) n -> p c n", p=128))
            pos = lnbc.tile([128, D], F32, tag="lnbc", name="pos")
            nc.sync.dma_start(out=pos[:], in_=pos_d)

            ltrow = ag.tile([1, D], F32)
            nc.vector.scalar_tensor_tensor(
                ltrow[:], rows_f[:, FO["ltw"]:FO["ltw"] + D], lead[:],
                rows_f[:, FO["ltb"]:FO["ltb"] + D], ALU.mult, ALU.add)
            ltrow_r = ag.tile([1, D], F32R)
            nc.vector.tensor_copy(ltrow_r[:], ltrow[:])
            l100 = ag.tile([1, 1], F32)
            nc.vector.tensor_scalar_mul(l100[:], lead[:], 1.0 / RES)
            mrow = ag.tile([1, NSTEPS], F32)
            nc.vector.tensor_scalar(
                mrow[:], rows_f[:, FO["steps"]:FO["steps"] + NSTEPS], l100[:],
                None, ALU.subtract)
            nc.vector.tensor_mul(mrow[:], mrow[:], mrow[:])
            nc.vector.tensor_scalar(mrow[:], mrow[:], 0.25, None, ALU.is_lt)
            mrow_r = ag.tile([1, NSTEPS], F32R)
            nc.vector.tensor_copy(mrow_r[:], mrow[:])
            mask_ps = pt.tile([128, NSTEPS], F32, tag="pt", name="mask_ps")
            nc.tensor.matmul(mask_ps[:], ones_r[:], mrow_r[:], start=True,
                             stop=True)
            nc.vector.tensor_copy(mask[:], mask_ps[:])

            qps = pm.tile([1, 1024], F32, tag="pm", name="q_ps")
            for nb in range(2):
                wh = load_w(aggq_d, f"aggq{nb}", nb * 512)
                sl = qps[:, nb * 512:(nb + 1) * 512]
                for c in range(KC):
                    nc.tensor.matmul(sl, vq[:, c, :], wh[:, c, :],
                                     start=(c == 0), stop=(c == KC - 1))
            qrow = ag.tile([1, D], F32)
            nc.vector.tensor_add(qrow[:], qps[0:1, :],
                                 rows_f[:, FO["aggqb"]:FO["aggqb"] + D])
            nc.vector.tensor_scalar_mul(qrow[:], qrow[:], SC)
            qrow_r = ag.tile([1, D], F32R)
            nc.vector.tensor_copy(qrow_r[:], qrow[:])
            qb_ps = pm.tile([128, 1024], F32, tag="pm", name="qb_ps")
            for nb in range(2):
                nc.tensor.matmul(qb_ps[:, nb * 512:(nb + 1) * 512], ones_r[:],
                                 qrow_r[:, nb * 512:(nb + 1) * 512],
                                 start=True, stop=True)
            qb = ag.tile([128, D], F32)
            nc.vector.tensor_copy(qb[:], qb_ps[:])

            s_all = ag.tile([128, NH, V], F32)
            v_nat = ag.tile([TPC, V, D], F32)
            tmp_s = ag.tile([TPC, D], F32)
            for v in range(V):
                tokb_v = rowp.tile([1, 2048], F32R, tag="rowp",
                                   name=f"tokb{v}")
                nc.sync.dma_start(out=tokb_v[:, :D],
                                  in_=tokb_d[:, v * D:(v + 1) * D])
                tokT = ag.tile([128, KC, 128], F32R, tag="tokT", bufs=2,
                               name=f"tokT{v}")
                for half in range(2):
                    pw_vh = ag.tile([4, 512], F32R, tag="pwv", bufs=2,
                                    name=f"pw{v}{half}")
                    nc.sync.dma_start(
                        out=pw_vh[:],
                        in_=pw_d[:, v * D + half * 512:v * D + (half + 1) * 512])
                    ps = pt.tile([128, 512], F32, tag="pt",
                                 name=f"tokps{v}{half}")
                    for j in range(4):
                        jj = half * 4 + j
                        sl = ps[:, j * 128:(j + 1) * 128]
                        nc.tensor.matmul(
                            sl, tokb_v[:, jj * 128:(jj + 1) * 128], ones_r[:],
                            start=True, stop=False)
                        nc.tensor.matmul(
                            sl, pw_vh[:, j * 128:(j + 1) * 128],
                            pat[:, v * TPC:(v + 1) * TPC],
                            start=False, stop=True)
                    nc.vector.tensor_copy(
                        tokT[:, half * 4:(half + 1) * 4, :].rearrange(
                            "p c t -> p (c t)"), ps[:])
                pk = mm_opt_a(tokT, aggk_d, r5("aggkb", 0, D), f"aggk_mm{v}")
                k_v = ag.tile([TPC, D], F32, tag="k_v", bufs=2, name=f"k_v{v}")
                nc.vector.tensor_copy(k_v[:], pk[:])
                pv = mm_opt_a(tokT, aggv_d, r5("aggvb", 0, D), f"aggv_mm{v}")
                nc.vector.tensor_copy(v_nat[:, v, :], pv[:])
                nc.vector.tensor_mul(tmp_s[:], k_v[:], qb[:])
                nc.vector.reduce_sum(
                    s_all[:, :, v:v + 1],
                    tmp_s[:].rearrange("p (h d) -> p h d", d=HD), axis=AX.X)

            smx = ag.tile([128, NH], F32)
            nc.vector.reduce_max(smx[:], s_all[:], axis=AX.X)
            sh = ag.tile([128, NH, V], F32)
            nc.vector.tensor_sub(sh[:], s_all[:],
                                 smx[:].unsqueeze(2).broadcast_to([128, NH, V]))
            ex = ag.tile([128, NH, V], F32)
            nc.scalar.activation(ex[:], sh[:], AF.Exp)
            den = ag.tile([128, NH], F32)
            nc.vector.reduce_sum(den[:], ex[:], axis=AX.X)
            rec = ag.tile([128, NH], F32)
            nc.vector.reciprocal(rec[:], den[:])
            attw = ag.tile([128, NH, V], F32)
            nc.vector.tensor_mul(attw[:], ex[:],
                                 rec[:].unsqueeze(2).broadcast_to([128, NH, V]))
            agg = ag.tile([TPC, D], F32)
            agg3 = agg[:].rearrange("p (h d) -> p h d", d=HD)
            tmp3 = tmp_s[:].rearrange("p (h d) -> p h d", d=HD)
            for v in range(V):
                vv = v_nat[:, v, :].rearrange("p (h d) -> p h d", d=HD)
                wv = attw[:, :, v:v + 1].broadcast_to([128, NH, HD])
                if v == 0:
                    nc.vector.tensor_mul(agg3, vv, wv)
                else:
                    nc.vector.tensor_mul(tmp3, vv, wv)
                    nc.vector.tensor_add(agg[:], agg[:], tmp_s[:])

            aggT = ag.tile([128, KC, 128], F32R)
            transpose_1024(aggT, agg[:], ident[:], "aggT")
            po = mm_opt_a(aggT, aggo_d, r5("aggob", 0, D), "aggo_mm",
                          extra_row=ltrow_r)
            nc.vector.tensor_add(xs[:], po[:], pos[:])

        # ================= Phase 2: depth loop
        with tc.tile_pool(name="odep", bufs=1) as od, \
                tc.tile_pool(name="odew", bufs=1) as odw:
            y_t = od.tile([TPC, D], F32, name="y_t")
            y2_t = od.tile([TPC, D], F32, name="y2_t")
            out_acc = od.tile([TPC, D], F32, name="out_acc")
            u_t = od.tile([TPC, D], F32, name="u_t")
            k_t = od.tile([TPC, D], F32, name="k_t")
            h1r = od.tile([TPC, D], BF16, name="h1r")

            for d in range(DEPTH):
                with tc.tile_pool(name=f"attn{d}", bufs=1) as at:
                    xsT = tpool.tile([128, KC, 128], F32R, tag="tp",
                                     name=f"xsT_{d}")
                    transpose_1024(xsT, xs[:], ident[:], f"xsT{d}")
                    qkb = rowp.tile([1, 2048], F32R, tag="rowp",
                                    name=f"qkb_{d}")
                    nc.sync.dma_start(out=qkb[:], in_=qkvb_d[d][:, :2048])
                    vbr = rowp.tile([1, 2048], F32R, tag="rowp",
                                    name=f"vbr_{d}")
                    nc.sync.dma_start(out=vbr[:, :D], in_=qkvb_d[d][:, 2048:])

                    def opt_b(dst, colbase, bias_ap, tag):
                        for half in range(2):
                            wh = load_w(qkv_d[d], f"{tag}{half}",
                                        colbase + half * 512)
                            ps = pt.tile([128, 512], F32, tag="pt",
                                         name=f"ps_{tag}{half}")
                            for j in range(4):
                                jj = half * 4 + j
                                sl = ps[:, j * 128:(j + 1) * 128]
                                nc.tensor.matmul(
                                    sl, bias_ap[:, jj * 128:(jj + 1) * 128],
                                    ones_r[:], start=True, stop=False)
                                for c in range(KC):
                                    nc.tensor.matmul(
                                        sl, wh[:, c, j * 128:(j + 1) * 128],
                                        xsT[:, c, :], start=False,
                                        stop=(c == KC - 1))
                            nc.vector.tensor_copy(
                                dst[:, half * 4:(half + 1) * 4, :].rearrange(
                                    "p c t -> p (c t)"), ps[:])

                    kTl = at.tile([128, KC, 128], BF16, name=f"kTl_{d}")
                    opt_b(kTl, D, qkb[:, D:2 * D], f"kw{d}")
                    kb = dram.tile([KC, 128, 128], BF16, name=f"kb_{d}")
                    nc.sync.dma_start(out=kb[:].rearrange("c p t -> p c t"),
                                      in_=kTl[:])
                    kg = dram.tile([GS, KC, 128, 128], BF16, name=f"kg_{d}")
                    nc.gpsimd.collective_compute(
                        "AllGather", ALU.bypass, replica_groups=rg,
                        ins=[kb[:].opt()], outs=[kg[:].opt()])

                    pv2 = pm.tile([128, 1024], F32, tag="pm", name=f"v_mm{d}")
                    for nb in range(2):
                        wh = load_w(qkv_d[d], f"vw{d}{nb}", 2 * D + nb * 512)
                        sl = pv2[:, nb * 512:(nb + 1) * 512]
                        nc.tensor.matmul(
                            sl, ones_r[:],
                            vbr[:, nb * 512:(nb + 1) * 512],
                            start=True, stop=False)
                        for c in range(KC):
                            nc.tensor.matmul(sl, xsT[:, c, :], wh[:, c, :],
                                             start=False, stop=(c == KC - 1))
                    v_loc = at.tile([TPC, D], BF16, name=f"vloc_{d}")
                    nc.vector.tensor_copy(v_loc[:], pv2[:])
                    vb = dram.tile([TPC, D], BF16, name=f"vb_{d}")
                    nc.sync.dma_start(out=vb[:], in_=v_loc[:])
                    vg = dram.tile([GS, TPC, D], BF16, name=f"vg_{d}")
                    nc.gpsimd.collective_compute(
                        "AllGather", ALU.bypass, replica_groups=rg,
                        ins=[vb[:].opt()], outs=[vg[:].opt()])

                    qT = at.tile([128, KC, 128], BF16, name=f"qT_{d}")
                    opt_b(qT, 0, qkb[:, 0:D], f"qw{d}")

                    kT_full = at.tile([128, KC, GS * 128], BF16,
                                      name=f"ktf_{d}")
                    for c in range(KC):
                        nc.sync.dma_start(
                            out=kT_full[:, c, :].rearrange(
                                "p (g t) -> p g t", g=GS),
                            in_=kg[:, c, :, :].rearrange("g p t -> p g t"))
                    v_full = at.tile([128, GS, D], BF16, name=f"vf_{d}")
                    nc.sync.dma_start(out=v_full[:],
                                      in_=vg[:].rearrange("g p n -> p g n"))

                    attnT = at.tile([128, KC, 128], F32R, name=f"attnT_{d}")
                    for hp in range(NH // 2):
                        avp = pt.tile([128, 128], F32, tag="pt",
                                      name=f"av_{d}_{hp}")
                        for hh in range(2):
                            h = hp * 2 + hh
                            c, half = h // 2, h % 2
                            p0 = 64 * half
                            scp = pt.tile([128, 512], F32, tag="pt",
                                          name=f"sc_{d}_{h}")
                            nc.tensor.matmul(
                                scp[:], qT[p0:p0 + 64, c, :],
                                kT_full[p0:p0 + 64, c, :], start=True,
                                stop=True, tile_position=(p0, 0))
                            smx2 = sb2.tile([128, 1], F32, tag="smx2",
                                            name=f"smx_{d}_{h}")
                            nc.vector.reduce_max(smx2[:], scp[:], axis=AX.X)
                            nc.vector.tensor_scalar_mul(smx2[:], smx2[:], -SC)
                            expn = sb2.tile([128, 512], F32, tag="expn",
                                            name=f"expn_{d}_{h}")
                            dn = sb2.tile([128, 1], F32, tag="dn",
                                          name=f"dn_{d}_{h}")
                            nc.scalar.activation(expn[:], scp[:], AF.Exp,
                                                 bias=smx2[:], scale=SC,
                                                 accum_out=dn[:])
                            rc = sb2.tile([128, 1], F32, tag="rc",
                                          name=f"rc_{d}_{h}")
                            nc.vector.reciprocal(rc[:], dn[:])
                            expr = sb2.tile([128, 512], BF16, tag="expr",
                                            name=f"expr_{d}_{h}")
                            nc.vector.tensor_scalar_mul(expr[:], expn[:],
                                                        rc[:])
                            etp = pt.tile([128, 512], BF16, tag="pt",
                                          name=f"etp_{d}_{h}")
                            for c2 in range(GS):
                                nc.tensor.transpose(
                                    etp[:, c2 * 128:(c2 + 1) * 128],
                                    expr[:, c2 * 128:(c2 + 1) * 128],
                                    identb[:])
                            expT = sb2.tile([128, 512], BF16, tag="expT",
                                            name=f"expT_{d}_{h}")
                            nc.vector.tensor_copy(expT[:], etp[:])
                            out_sl = avp[hh * 64:(hh + 1) * 64, :]
                            for c2 in range(GS):
                                nc.tensor.matmul(
                                    out_sl,
                                    v_full[:, c2, h * HD:(h + 1) * HD],
                                    expT[:, c2 * 128:(c2 + 1) * 128],
                                    start=(c2 == 0), stop=(c2 == GS - 1),
                                    tile_position=(0, hh * 64))
                        nc.vector.tensor_copy(attnT[:, hp, :], avp[:])

                    po2 = mm_opt_a(attnT, ow_d[d], r5(f"ob{d}", 0, D),
                                   f"oproj{d}")
                    nc.vector.tensor_add(z_t[:], po2[:], xs[:])
                    g1 = ln_bc(LN[f"n1_{d}"], f"n1g{d}")
                    b1 = ln_bc(LN[f"n1b_{d}"], f"n1b{d}")
                    ln_tok(xs[:], z_t[:], g1[:], b1[:])

                # ---------------- ODE block
                b12 = rload(f"b1b2_{d}", 2048, f"b12_{d}")
                w1t = odw.tile([128, KC, D], BF16, tag="w1", name=f"w1t_{d}")
                w2t = odw.tile([128, KC, D], BF16, tag="w2", name=f"w2t_{d}")
                for t_, d_ in ((w1t, w1_d[d]), (w2t, w2_d[d])):
                    src = d_.rearrange("(c p) n -> p c n", p=128)
                    for c in range(KC):
                        nc.sync.dma_start(out=t_[:, c, :], in_=src[:, c, :])
                og = ln_bc(LN[f"og_{d}"], f"og{d}")
                ob = ln_bc(LN[f"ob_{d}"], f"ob{d}")

                nc.vector.tensor_copy(y_t[:], xs[:])
                nc.vector.tensor_copy(out_acc[:], xs[:])
                y, y2 = y_t, y2_t

                def f_eval(src, dst, tag):
                    uT = tpool.tile([128, KC, 128], BF16, tag="uT",
                                    name=f"uT_{tag}")
                    transpose_1024(uT, src[:], ident[:], f"u{tag}")
                    ph = pm.tile([128, 1024], F32, tag="pm", name=f"mm1_{tag}")
                    for nb in range(2):
                        sl = ph[:, nb * 512:(nb + 1) * 512]
                        nc.tensor.matmul(sl, ones_r[:],
                                         b12[:, nb * 512:(nb + 1) * 512],
                                         start=True, stop=False)
                        for c in range(KC):
                            nc.tensor.matmul(
                                sl, uT[:, c, :],
                                w1t[:, c, nb * 512:(nb + 1) * 512],
                                start=False, stop=(c == KC - 1))
                        nc.scalar.activation(h1r[:, nb * 512:(nb + 1) * 512],
                                             sl, AF.Relu)
                    h1T = tpool.tile([128, KC, 128], BF16, tag="h1T",
                                     name=f"h1T_{tag}")
                    transpose_1024(h1T, h1r[:], identb[:], f"h{tag}")
                    pz = pm.tile([128, 1024], F32, tag="pm", name=f"mm2_{tag}")
                    for nb in range(2):
                        sl = pz[:, nb * 512:(nb + 1) * 512]
                        nc.tensor.matmul(
                            sl, ones_r[:],
                            b12[:, D + nb * 512:D + (nb + 1) * 512],
                            start=True, stop=False)
                        for c in range(KC):
                            nc.tensor.matmul(
                                sl, h1T[:, c, :],
                                w2t[:, c, nb * 512:(nb + 1) * 512],
                                start=False, stop=(c == KC - 1))
                    nc.vector.tensor_add(z_t[:], pz[:], src[:])
                    ln_tok(dst[:], z_t[:], og[:], ob[:])

                for s in range(NSTEPS):
                    f_eval(y, k_t, f"d{d}s{s}a")
                    nc.vector.scalar_tensor_tensor(
                        y2[:], k_t[:], RES / 6.0, y[:], ALU.mult, ALU.add)
                    nc.vector.scalar_tensor_tensor(
                        u_t[:], k_t[:], 0.5 * RES, y[:], ALU.mult, ALU.add)
                    f_eval(u_t, k_t, f"d{d}s{s}b")
                    nc.vector.scalar_tensor_tensor(
                        y2[:], k_t[:], RES / 3.0, y2[:], ALU.mult, ALU.add)
                    nc.vector.scalar_tensor_tensor(
                        u_t[:], k_t[:], 0.5 * RES, y[:], ALU.mult, ALU.add)
                    f_eval(u_t, k_t, f"d{d}s{s}c")
                    nc.vector.scalar_tensor_tensor(
                        y2[:], k_t[:], RES / 3.0, y2[:], ALU.mult, ALU.add)
                    nc.vector.scalar_tensor_tensor(
                        u_t[:], k_t[:], RES, y[:], ALU.mult, ALU.add)
                    f_eval(u_t, k_t, f"d{d}s{s}d")
                    nc.vector.scalar_tensor_tensor(
                        y2[:], k_t[:], RES / 6.0, y2[:], ALU.mult, ALU.add)
                    nc.vector.tensor_sub(z_t[:], y2[:], out_acc[:])
                    nc.vector.scalar_tensor_tensor(
                        out_acc[:], z_t[:], mask[:, s:s + 1], out_acc[:],
                        ALU.mult, ALU.add)
                    y, y2 = y2, y

                nc.vector.tensor_add(z_t[:], xs[:], out_acc[:])
                g2 = ln_bc(LN[f"n2_{d}"], f"n2g{d}")
                b2 = ln_bc(LN[f"n2b_{d}"], f"n2b{d}")
                ln_tok(xs[:], z_t[:], g2[:], b2[:])

        # ================= Phase 3: head
        with tc.tile_pool(name="headp", bufs=1) as hp_:
            ng = ln_bc(LN["normg"], "normg")
            nbt = ln_bc(LN["normb"], "normb")
            ln_tok(z_t[:], xs[:], ng[:], nbt[:])
            headb = rload("headb", 2048, "headb")
            hT = hp_.tile([128, KC, 128], F32R, name="hT")
            transpose_1024(hT, z_t[:], ident[:], "headT")
            ph1 = mm_opt_a(hT, h1_d, headb[:, 0:D], "h1_mm")
            g1r = hp_.tile([TPC, D], F32R, name="g1r")
            for nb in range(2):
                nc.scalar.activation(g1r[:, nb * 512:(nb + 1) * 512],
                                     ph1[:, nb * 512:(nb + 1) * 512], AF.Gelu)
            g1T = hp_.tile([128, KC, 128], F32R, name="g1T")
            transpose_1024(g1T, g1r[:], identr[:], "g1T")
            ph2 = mm_opt_a(g1T, h2_d, headb[:, D:2 * D], "h2_mm")
            g2r = hp_.tile([TPC, D], F32R, name="g2r")
            for nb in range(2):
                nc.scalar.activation(g2r[:, nb * 512:(nb + 1) * 512],
                                     ph2[:, nb * 512:(nb + 1) * 512], AF.Gelu)
            g2T = hp_.tile([128, KC, 128], F32R, name="g2T")
            transpose_1024(g2T, g2r[:], identr[:], "g2T")
            hfb = rload("hfb", NOUT, "hfb")
            pf = pt.tile([128, NOUT], F32, tag="pt", name="hf_ps")
            nc.tensor.matmul(pf[:], ones_r[:], hfb[:, :NOUT], start=True,
                             stop=False)
            for c in range(KC):
                nc.tensor.matmul(pf[:], g2T[:, c, :], hfw[:, c, :],
                                 start=False, stop=(c == KC - 1))
            out_sb = hp_.tile([TPC, NOUT], F32, name="out_sb")
            nc.vector.tensor_copy(out_sb[:], pf[:])
            nc.sync.dma_start(out=y_d, in_=out_sb[:])

    nc.compile()
    return nc


# ---------------------------------------------------------------- host side
def _prep_inputs(inputs):
    f32 = np.float32
    bf = ml_dtypes.bfloat16
    x = np.asarray(inputs["x"], f32)
    lead_times = np.asarray(inputs["lead_times"], f32)
    g = lambda n: np.ascontiguousarray(np.asarray(inputs[n], f32))

    patches = x.reshape(B, V, H // P, P, W // P, P).transpose(
        0, 1, 2, 4, 3, 5).reshape(B, V, L, P * P)

    agg_in_w, agg_in_b = g("agg_in_w"), g("agg_in_b")

    rows_r = np.zeros((1, RRN), f32)
    rows_r[0, RR["aggkb"]:RR["aggkb"] + D] = agg_in_b[D:2 * D]
    rows_r[0, RR["aggvb"]:RR["aggvb"] + D] = agg_in_b[2 * D:]
    rows_r[0, RR["aggob"]:RR["aggob"] + D] = g("agg_out_b")
    rows_r[0, RR["ob0"]:RR["ob0"] + D] = g("blk_out_b")[0]
    rows_r[0, RR["ob1"]:RR["ob1"] + D] = g("blk_out_b")[1]
    for d in range(DEPTH):
        rows_r[0, RR[f"b1b2_{d}"]:RR[f"b1b2_{d}"] + D] = g("ode_b1")[d]
        rows_r[0, RR[f"b1b2_{d}"] + D:RR[f"b1b2_{d}"] + 2 * D] = g("ode_b2")[d]
    rows_r[0, RR["headb"]:RR["headb"] + D] = g("h1_b")
    rows_r[0, RR["headb"] + D:RR["headb"] + 2 * D] = g("h2_b")
    rows_r[0, RR["hfb"]:RR["hfb"] + NOUT] = g("hf_b")

    rows_f = np.zeros((1, FN), f32)
    rows_f[0, FO["ltw"]:FO["ltw"] + D] = g("lt_w")[:, 0]
    rows_f[0, FO["ltb"]:FO["ltb"] + D] = g("lt_b")
    rows_f[0, FO["aggqb"]:FO["aggqb"] + D] = agg_in_b[:D]
    rows_f[0, FO["steps"]:FO["steps"] + NSTEPS] = np.arange(
        1, NSTEPS + 1, dtype=f32)

    ln_rows = np.zeros((14, D), f32)
    for i, val in enumerate([
            g("blk_n1_g")[0], g("blk_n1_b")[0], g("blk_n2_g")[0],
            g("blk_n2_b")[0], g("blk_n1_g")[1], g("blk_n1_b")[1],
            g("blk_n2_g")[1], g("blk_n2_b")[1],
            g("ode_ng")[0], g("ode_nb")[0], g("ode_ng")[1], g("ode_nb")[1],
            g("norm_g"), g("norm_b")]):
        ln_rows[i] = val

    shared = {
        "pw": np.ascontiguousarray(
            g("patch_w").transpose(2, 0, 1).reshape(4, V * D)),
        "aggk": np.ascontiguousarray(agg_in_w[D:2 * D].T),
        "aggv": np.ascontiguousarray(agg_in_w[2 * D:].T),
        "aggq": np.ascontiguousarray(agg_in_w[:D].T),
        "aggo": np.ascontiguousarray(g("agg_out_w").T),
        "vq": np.ascontiguousarray(g("var_query").reshape(D, 1)),
        "qkv0": np.ascontiguousarray(g("blk_qkv_w")[0].T),
        "qkv1": np.ascontiguousarray(g("blk_qkv_w")[1].T),
        "ow0": np.ascontiguousarray(g("blk_out_w")[0].T),
        "ow1": np.ascontiguousarray(g("blk_out_w")[1].T),
        "w1_0": g("ode_w1")[0].T.astype(bf),
        "w2_0": g("ode_w2")[0].T.astype(bf),
        "w1_1": g("ode_w1")[1].T.astype(bf),
        "w2_1": g("ode_w2")[1].T.astype(bf),
        "h1w": np.ascontiguousarray(g("h1_w").T),
        "h2w": np.ascontiguousarray(g("h2_w").T),
        "hfw": np.ascontiguousarray(g("hf_w").T),
        "rows_r": rows_r,
        "tokb": (g("patch_b") + g("var_embed")).reshape(1, V * D),
        "qkvb0": g("blk_qkv_b")[0].reshape(1, 3 * D),
        "qkvb1": g("blk_qkv_b")[1].reshape(1, 3 * D),
        "rows_f": rows_f, "lnrows": ln_rows,
        "ident": np.eye(128, dtype=f32), "identr": np.eye(128, dtype=f32),
        "identb": np.eye(128, dtype=f32).astype(bf),
        "ones_r": np.ones((1, 128), f32),
    }

    in_maps = []
    for c in range(NCORES):
        b, q = c // GS, c % GS
        sl = slice(q * TPC, (q + 1) * TPC)
        m = dict(shared)
        m["patches"] = np.ascontiguousarray(
            patches[b, :, sl, :].transpose(2, 0, 1).reshape(4, V * TPC))
        m["lead"] = lead_times[b].reshape(1, 1)
        m["pos"] = np.ascontiguousarray(g("pos_embed")[sl])
        in_maps.append(m)
    return in_maps


def kernel(**inputs):
    if "nc" not in _CACHE:
        _CACHE["nc"] = build_program()
    nc = _CACHE["nc"]
    in_maps = _prep_inputs(inputs)
    res = bass_utils.run_bass_kernel_spmd(nc, in_maps,
                                          core_ids=list(range(NCORES)))
    y_all = np.empty((B, L, NOUT), np.float32)
    for c in range(NCORES):
        b, q = c // GS, c % GS
        y_all[b, q * TPC:(q + 1) * TPC] = res.results[c]["y"]
    y = y_all.reshape(B, H // P, W // P, P, P, V)
    y = np.einsum("nhwpqc->nchpwq", y).reshape(B, V, H, W)
    return y.astype(np.float32)
